# revision 21
# baseline (speedup 1.0000x reference)
"""KStepRGCN Trainium2 kernel: 8-core SPMD Bass/Tile implementation.

Sharding: nodes partitioned into 8 dst-slices (graph-partition style).
Each core aggregates messages for its dst-slice via pipelined dma_gather
(bf16 rows from a replicated node-feature table) + PE one-hot segment-sum
matmuls. The one-hot S matrices are generated on-chip (DVE iota-compare
against per-edge column indices) with the mean divisor folded into the
one-hot values, so the root/bias terms accumulate into the same PSUM
group. Between layers the updated slices are AllGathered into the next
table.
"""

import sys

sys.path.insert(0, "/opt/trn_rl_repo")

import os

import numpy as np
import ml_dtypes

BF16 = ml_dtypes.bfloat16

# ablation switches (benchmarking only — break correctness)
DBG_NOGATHER = os.environ.get("DBG_NOGATHER", "0") == "1"
DBG_NOWAIT = os.environ.get("DBG_NOWAIT", "0") == "1"
DBG_NOMM = os.environ.get("DBG_NOMM", "0") == "1"
DBG_NOSGEN = os.environ.get("DBG_NOSGEN", "0") == "1"

# problem constants (hardcoded per harness contract)
N, E, D, R, B, K = 50000, 600000, 128, 3, 3, 3
NCORES = 8
LO_LIMIT = 32768
SEGC = int(os.environ.get("SEGC", "16"))  # chunks per gather segment
SINGLE_PACKET = os.environ.get("SP", "0") == "1"
NQ = int(os.environ.get("NQ", "1"))       # SWDGE queues (>1 crashes runtime)
PDEPTH = int(os.environ.get("PDEPTH", "0"))  # >0 hangs this runtime
MSG_BUFS = int(os.environ.get("MBUFS", "6"))   # in-flight gather segments/stream
S_BUFS = int(os.environ.get("SBUFS", "6"))
LOOKAHEAD = int(os.environ.get("LA", "4"))     # segments emitted ahead of consumer


class Cfg:
    def __init__(self, n=N, e=E, ncores=NCORES):
        assert n % ncores == 0
        self.n, self.e, self.ncores = n, e, ncores
        self.ns = n // ncores                 # real nodes per slice
        self.tpc = (self.ns + 127) // 128     # col tiles per relation
        self.nsp = self.tpc * 128             # padded slice
        self.trows = ncores * self.nsp        # table rows
        self.nblk = R * self.tpc              # psum blocks per layer


def _wrap_idx(idx_flat, nseg):
    """[nseg*SEGC*128] -> wrapped [128, nseg*SEGC*8] int16."""
    tot = nseg * SEGC
    return np.tile(
        idx_flat.reshape(nseg, SEGC * 8, 16).transpose(0, 2, 1)
        .reshape(nseg, 16, SEGC * 8).transpose(1, 0, 2).reshape(16, tot * 8),
        (8, 1)).astype(np.int16)


def _preprocess(cfg, edge_index, edge_attr):
    """Build the uniform (cross-core) static schedule + per-core host data.

    Schedule: per (block, stream) chunk counts = max over cores, chunks
    packed densely per stream in block order into SEGC-chunk gather
    segments.
    """
    src = np.asarray(edge_index[0], dtype=np.int64)
    dst = np.asarray(edge_index[1], dtype=np.int64)
    attr = np.asarray(edge_attr, dtype=np.int64)
    ns, nsp, tpc, nc_, nblk = cfg.ns, cfg.nsp, cfg.tpc, cfg.ncores, cfg.nblk

    deg_total = np.bincount(dst, minlength=cfg.n)
    inv_cnt = 1.0 / np.maximum(deg_total, 1).astype(np.float32)

    # --- per-core node permutation: snake-balance total degree across bins
    perms = []
    for c in range(nc_):
        deg_local = deg_total[c * ns:(c + 1) * ns]
        order = np.argsort(-deg_local, kind="stable")
        i = np.arange(ns)
        g, o = i // tpc, i % tpc
        b = np.where(g % 2 == 0, o, tpc - 1 - o)      # snake over bins
        perm = np.empty(ns, dtype=np.int64)
        perm[order] = b * 128 + g
        perms.append(perm)

    row_of = np.empty(cfg.n, dtype=np.int64)
    for c in range(nc_):
        row_of[c * ns:(c + 1) * ns] = c * nsp + perms[c]

    lo_lim = min(LO_LIMIT, cfg.trows)
    hi_rows = cfg.trows - lo_lim
    nstreams = 2 if hi_rows > 0 else 1

    # --- per-core edge bucketing by (block, stream)
    core_of = dst // ns
    edges_pc = []   # per core per stream: (row_rel, bl, colw, invc_e) sorted by bl
    cnt = np.zeros((nc_, nblk, 2), dtype=np.int64)
    for c in range(nc_):
        m = core_of == c
        s_c, v_c, r_c = src[m], dst[m] - c * ns, attr[m]
        pos = perms[c][v_c]
        bl = r_c * tpc + pos // 128
        colw = pos % 128
        row = row_of[s_c]
        ive = inv_cnt[dst[m]]
        is_lo = row < lo_lim
        parts = []
        for sidx, (sel, base) in enumerate(((is_lo, 0), (~is_lo, lo_lim))):
            blv, rv, cw, iv = bl[sel], row[sel] - base, colw[sel], ive[sel]
            # sort by (bucket, src row): ascending rows per chunk give the
            # SDMA engines near-sequential HBM reads within each gather
            order = np.lexsort((rv, blv))
            blv, rv, cw, iv = blv[order], rv[order], cw[order], iv[order]
            np.add.at(cnt[c, :, sidx], blv, 1)
            parts.append((rv, blv, cw, iv))
        edges_pc.append(parts)

    # --- uniform chunk counts per (block, stream): max over cores
    nch = np.ceil(cnt / 128.0).astype(np.int64).max(axis=0)  # [nblk, 2]
    # guard: every block needs >= 1 chunk so its psum group is written
    empty = nch.sum(axis=1) == 0
    nch[empty, 0] = 1
    if nstreams == 1:
        nch[:, 1] = 0

    qoff = np.zeros((nblk, 2), dtype=np.int64)  # chunk offset within stream
    qoff[:, 0] = np.cumsum(nch[:, 0]) - nch[:, 0]
    qoff[:, 1] = np.cumsum(nch[:, 1]) - nch[:, 1]
    nch_s = [int(nch[:, 0].sum()), int(nch[:, 1].sum())]
    nseg = [(nch_s[0] + SEGC - 1) // SEGC,
            (nch_s[1] + SEGC - 1) // SEGC if nch_s[1] else 0]

    # segment emission order: by (first-use block, stream)
    seg_first_use = []
    for s in range(2):
        for g in range(nseg[s]):
            q0 = g * SEGC
            # first block whose chunk range covers q0 (or follows it)
            fub = int(np.searchsorted(qoff[:, s] + nch[:, s], q0 + 1))
            seg_first_use.append((fub, s, g))
    seg_order = [(s, g) for _, s, g in sorted(seg_first_use)]

    # --- per-core tensors: wrapped idx + cv + invce per stream
    per_core = []
    for c in range(nc_):
        dat = {}
        for s in range(nstreams):
            if nseg[s] == 0:
                continue
            tot = nseg[s] * SEGC
            idx_flat = np.zeros(tot * 128, dtype=np.int16)
            cv = np.full((128, tot), 255.0, dtype=np.float32)
            ive_a = np.ones((128, tot), dtype=np.float32)
            rv, blv, cw, iv = edges_pc[c][s]
            if len(rv):
                start = np.zeros(nblk, dtype=np.int64)
                cnt_c = np.bincount(blv, minlength=nblk)
                start[1:] = np.cumsum(cnt_c)[:-1]
                rank = np.arange(len(blv)) - start[blv]
                q = qoff[blv, s] + rank // 128
                p = rank % 128
                idx_flat[q * 128 + p] = rv.astype(np.int16)
                cv[p, q] = cw
                ive_a[p, q] = iv
            dat[f"idx{s}"] = _wrap_idx(idx_flat, nseg[s])
            dat[f"cv{s}"] = cv.astype(np.float32)
            dat[f"ivc{s}"] = ive_a.astype(np.float32)
        per_core.append(dat)

    sched = dict(nch=nch, qoff=qoff, nseg=nseg, lo_lim=lo_lim,
                 hi_rows=hi_rows, nstreams=nstreams, seg_order=seg_order)
    return sched, per_core, perms, inv_cnt


def _build_program(cfg, sched, k_layers=K, prelu_a=0.25, n_iter=1):
    from concourse import bacc, mybir
    import concourse.tile as tile

    f32, bf16, i16 = mybir.dt.float32, mybir.dt.bfloat16, mybir.dt.int16
    Alu = mybir.AluOpType
    Act = mybir.ActivationFunctionType
    tpc, nsp, nblk, trows = cfg.tpc, cfg.nsp, cfg.nblk, cfg.trows
    nch, qoff = sched["nch"], sched["qoff"]
    nseg, nstreams = sched["nseg"], sched["nstreams"]
    lo_lim, hi_rows = sched["lo_lim"], sched["hi_rows"]
    seg_order = sched["seg_order"]

    nc = bacc.Bacc("TRN2", target_bir_lowering=False, debug=False,
                   num_devices=cfg.ncores, num_swdge_queues=NQ)

    # --- IO tensors
    x_table = nc.dram_tensor("x_table", [trows, D], bf16, kind="ExternalInput")
    x_own = nc.dram_tensor("x_own", [128, nsp], f32, kind="ExternalInput")
    w_sw = nc.dram_tensor("w_sw", [128, k_layers * R * D], bf16,
                          kind="ExternalInput")
    root_sw = nc.dram_tensor("root_sw", [128, k_layers * D], bf16,
                             kind="ExternalInput")
    bias_in = nc.dram_tensor("bias_in", [1, k_layers * D], bf16,
                             kind="ExternalInput")
    ident_in = nc.dram_tensor("ident_in", [128, 128], f32, kind="ExternalInput")
    iota_in = nc.dram_tensor("iota_in", [128, 128], bf16, kind="ExternalInput")
    idx_in, cv_in, ivc_in = [None, None], [None, None], [None, None]
    for s in range(nstreams):
        if nseg[s]:
            idx_in[s] = nc.dram_tensor(f"idx{s}", [128, nseg[s] * SEGC * 8],
                                       i16, kind="ExternalInput")
            cv_in[s] = nc.dram_tensor(f"cv{s}", [128, nseg[s] * SEGC], f32,
                                      kind="ExternalInput")
            ivc_in[s] = nc.dram_tensor(f"ivc{s}", [128, nseg[s] * SEGC], f32,
                                       kind="ExternalInput")
    out_own = nc.dram_tensor("out_own", [nsp, D], f32, kind="ExternalOutput")

    # internal tables for AllGather
    ag_in = nc.dram_tensor("ag_in", [nsp, D], bf16, kind="Internal")
    tables = [x_table]
    for i in range(k_layers - 1):
        tables.append(nc.dram_tensor(f"table{i + 1}", [trows, D], bf16,
                                     kind="Internal", addr_space="Shared"))

    rg = [list(range(cfg.ncores))]

    from contextlib import ExitStack

    with tile.TileContext(nc) as tc, ExitStack() as ctx:
        const = ctx.enter_context(tc.tile_pool(name="const", bufs=1))
        w_t = const.tile([128, k_layers * R * D], bf16, tag="w")
        root_t = const.tile([128, k_layers * D], bf16, tag="root")
        bias_t = const.tile([1, k_layers * D], bf16, tag="bias")
        ones_t = const.tile([1, 128], bf16, tag="ones")
        ident_t = const.tile([128, 128], f32, tag="ident")
        iota_t = const.tile([128, 128], bf16, tag="iota")
        h_own = const.tile([128, nsp], f32, tag="h_own")
        a_T = const.tile([128, nblk * 128], bf16, tag="a_T")
        hbf = const.tile([128, nsp], bf16, tag="hbf")
        idx_t, cv_t, ivc_t = [None, None], [None, None], [None, None]
        for s in range(nstreams):
            if nseg[s]:
                idx_t[s] = const.tile([128, nseg[s] * SEGC * 8], i16,
                                      name=f"idxt{s}", tag=f"ix{s}")
                cv_t[s] = const.tile([128, nseg[s] * SEGC], f32,
                                     name=f"cvt{s}", tag=f"cv{s}")
                ivc_t[s] = const.tile([128, nseg[s] * SEGC], f32,
                                      name=f"ivct{s}", tag=f"iv{s}")
                nc.sync.dma_start(idx_t[s][:], idx_in[s].ap())
                nc.sync.dma_start(cv_t[s][:], cv_in[s].ap())
                nc.sync.dma_start(ivc_t[s][:], ivc_in[s].ap())

        nc.sync.dma_start(w_t[:], w_sw.ap())
        nc.sync.dma_start(root_t[:], root_sw.ap())
        nc.sync.dma_start(bias_t[:], bias_in.ap())
        nc.sync.dma_start(ident_t[:], ident_in.ap())
        nc.sync.dma_start(iota_t[:], iota_in.ap())
        nc.vector.memset(ones_t[:], 1.0)

        msg_pools = [
            ctx.enter_context(tc.tile_pool(name=f"msg{s}", bufs=MSG_BUFS))
            for s in range(nstreams)]
        s_pools = [
            ctx.enter_context(tc.tile_pool(name=f"sp{s}", bufs=S_BUFS))
            for s in range(nstreams)]
        pblk = ctx.enter_context(tc.tile_pool(name="pblk", bufs=4,
                                              space="PSUM"))
        pout = ctx.enter_context(tc.tile_pool(name="pout", bufs=2,
                                              space="PSUM"))
        ptr_p = ctx.enter_context(tc.tile_pool(name="ptr", bufs=2,
                                               space="PSUM"))
        hT_pool = ctx.enter_context(tc.tile_pool(name="hT", bufs=2))

        # pipelined SWDGE gathers: rotating per-slot completion semaphores;
        # consumers (PE) wait on the slot sem, prep/trigger never wait for
        # data. (auto-trigger dma_gather crashes this runtime; staged
        # prepare_only + trigger works.)
        prep_sems = [ctx.enter_context(nc.semaphore(f"prep_sem{q}"))
                     for q in range(NQ)]
        slot_sems = [[ctx.enter_context(nc.semaphore(f"dsem{s}_{i}"))
                      for i in range(MSG_BUFS)] for s in range(nstreams)]
        # slot index tracks the msg pool's round-robin buffer assignment
        # (one tile() call per emission), so a slot sem never has two
        # outstanding gathers: prep of emission e waits (pool WAR dep) for
        # the consumers of emission e-MSG_BUFS, which waited on this sem.
        emis_count = [0, 0]
        glob_emis = [0]
        prep_counts = [0] * NQ
        pending_trig = []     # FIFO of (queue, prep_count, (s, seg))
        seg_slot = [{}, {}]   # (s, seg) -> (slot, use_idx) for current layer
        triggered = set()

        def emit_trigger_one():
            q, pc, key = pending_trig.pop(0)
            nc.gpsimd.wait_ge(prep_sems[q], pc)
            nc.gpsimd.trigger_dma(count=1, queue_num=q)
            triggered.add(key)

        def emit_gather(s, seg, mt, table):
            # software-pipelined desc-gen: prep segment e on queue e%NQ and
            # trigger segment e-PDEPTH, whose Q7 desc-gen overlapped the
            # last PDEPTH preps (one desc-gen context per SWDGE queue).
            if s == 0:
                in_ap = table.ap()[0:lo_lim, :]
            else:
                in_ap = table.ap()[lo_lim:trows, :]
            slot = emis_count[s] % MSG_BUFS
            uses = emis_count[s] // MSG_BUFS + 1
            emis_count[s] += 1
            sem = slot_sems[s][slot]
            q = glob_emis[0] % NQ
            glob_emis[0] += 1
            prep_counts[q] += 1
            seg_slot[s][seg] = (slot, uses)
            pending_trig.append((q, prep_counts[q], (s, seg)))
            with tc.tile_critical():
                nc.gpsimd.dma_gather(
                    out_ap=mt[:], in_ap=in_ap,
                    idxs_ap=idx_t[s][:, seg * SEGC * 8:(seg + 1) * SEGC * 8],
                    num_idxs=SEGC * 128, num_idxs_reg=SEGC * 128, elem_size=D,
                    prepare_only=True, sem=sem, queue_num=q,
                    single_packet=SINGLE_PACKET).then_inc(prep_sems[q], 1)
                while len(pending_trig) > PDEPTH:
                    emit_trigger_one()

        def flush_triggers(key=None):
            # fire pending triggers (all, or until `key` has been triggered)
            if not pending_trig or (key is not None and key in triggered):
                return
            with tc.tile_critical():
                while pending_trig and (key is None or key not in triggered):
                    emit_trigger_one()

        for it in range(n_iter):
            nc.sync.dma_start(h_own[:], x_own.ap())
            for k in range(k_layers):
                table = tables[k]
                tiles = {}
                waited = set()
                seg_slot[0].clear()
                seg_slot[1].clear()

                def emit_segment(s, seg):
                    mt = msg_pools[s].tile([128, SEGC, D], bf16, tag="m")
                    if not DBG_NOGATHER:
                        emit_gather(s, seg, mt, table)
                    else:
                        nc.vector.memset(mt[:, 0, 0:8], 0.0)
                    st = s_pools[s].tile([128, SEGC * 128], bf16, tag="s")
                    if not DBG_NOSGEN:
                        for j in range(SEGC):
                            q = seg * SEGC + j
                            nc.vector.tensor_scalar(
                                st[:, j * 128:(j + 1) * 128], iota_t[:],
                                cv_t[s][:, q:q + 1], ivc_t[s][:, q:q + 1],
                                Alu.is_equal, Alu.mult)
                    else:
                        nc.vector.memset(st[:, 0:8], 0.0)
                    tiles[(s, seg)] = (mt, st)

                emit_ptr = [0]

                def emit_ahead(upto_idx):
                    while emit_ptr[0] <= upto_idx and emit_ptr[0] < len(seg_order):
                        s, g = seg_order[emit_ptr[0]]
                        emit_segment(s, g)
                        emit_ptr[0] += 1

                seg_idx = {sg: i for i, sg in enumerate(seg_order)}

                # ---- segment-sum into a_T blocks
                for bl in range(nblk):
                    chunks = [(0, int(qoff[bl, 0]) + j)
                              for j in range(int(nch[bl, 0]))]
                    chunks += [(1, int(qoff[bl, 1]) + j)
                               for j in range(int(nch[bl, 1]))]
                    pb = pblk.tile([128, 128], f32, tag="pb")
                    n_mm = len(chunks)
                    for i, (s, q) in enumerate(chunks):
                        seg, pos = q // SEGC, q % SEGC
                        if (s, seg) not in waited:
                            # keep LOOKAHEAD gathers in flight ahead of the
                            # consumer (criticals chain globally, so the
                            # consumer-side wait-critical throttles emission)
                            emit_ahead(seg_idx[(s, seg)] + LOOKAHEAD)
                            if not (DBG_NOGATHER or DBG_NOWAIT):
                                flush_triggers((s, seg))
                                slot, uses = seg_slot[s][seg]
                                with tc.tile_critical():
                                    nc.tensor.wait_ge(slot_sems[s][slot],
                                                      16 * uses)
                            waited.add((s, seg))
                        mt, st = tiles[(s, seg)]
                        if not DBG_NOMM:
                            nc.tensor.matmul(
                                pb[:], lhsT=mt[:, pos, :],
                                rhs=st[:, pos * 128:(pos + 1) * 128],
                                start=(i == 0), stop=(i == n_mm - 1))
                    if not DBG_NOMM:
                        nc.scalar.activation(a_T[:, bl * 128:(bl + 1) * 128],
                                             pb[:], Act.Copy)

                # ---- transform per col-tile (root+bias fused in psum)
                def transpose_tile(t):
                    pt = ptr_p.tile([128, 128], f32, tag="pt")
                    nc.tensor.transpose(pt[:], h_own[:, t * 128:(t + 1) * 128],
                                        ident_t[:])
                    hT = hT_pool.tile([128, 128], bf16, tag="h")
                    nc.scalar.activation(hT[:], pt[:], Act.Copy)
                    return hT

                hT_next = transpose_tile(0)
                for t in range(tpc):
                    hT = hT_next
                    if t + 1 < tpc:
                        hT_next = transpose_tile(t + 1)
                    po = pout.tile([128, 128], f32, tag="po")
                    if not DBG_NOMM:
                        for r in range(R):
                            bl = r * tpc + t
                            nc.tensor.matmul(
                                po[:], lhsT=a_T[:, bl * 128:(bl + 1) * 128],
                                rhs=w_t[:, (k * R + r) * D:(k * R + r + 1) * D],
                                start=(r == 0), stop=False)
                    nc.tensor.matmul(po[:], lhsT=hT[:],
                                     rhs=root_t[:, k * D:(k + 1) * D],
                                     start=DBG_NOMM, stop=False)
                    nc.tensor.matmul(po[:], lhsT=ones_t[:],
                                     rhs=bias_t[:, k * D:(k + 1) * D],
                                     start=False, stop=True)
                    dst_sl = h_own[:, t * 128:(t + 1) * 128]
                    if k < k_layers - 1:
                        nc.scalar.activation(dst_sl, po[:], Act.Prelu,
                                             alpha=float(prelu_a))
                    else:
                        nc.scalar.activation(dst_sl, po[:], Act.Copy)

                if not DBG_NOGATHER:
                    flush_triggers()

                # ---- export: cast + AllGather (not after last layer)
                if k < k_layers - 1:
                    nc.vector.tensor_copy(hbf[:], h_own[:])
                    nc.sync.dma_start(
                        ag_in.ap().rearrange("(t p) f -> p t f", p=128),
                        hbf[:].rearrange("p (t f) -> p t f", f=D))
                    nc.gpsimd.collective_compute(
                        "AllGather", Alu.bypass, replica_groups=rg,
                        ins=[ag_in.ap()], outs=[tables[k + 1].ap()])

        nc.sync.dma_start(out_own.ap().rearrange("(t p) f -> p t f", p=128),
                          h_own[:].rearrange("p (t f) -> p t f", f=D))

    nc.compile()
    return nc


def _host_tensors(cfg, sched, per_core, perms, inv_cnt, x, basis, att, root,
                  bias, k_layers=K):
    """Build in_maps for all cores."""
    ns, nsp, tpc = cfg.ns, cfg.nsp, cfg.tpc
    nstreams, nseg = sched["nstreams"], sched["nseg"]
    W = np.einsum("krb,kbio->krio", att.astype(np.float32),
                  basis.astype(np.float32))[:k_layers]  # [k,R,D,D]
    root = root[:k_layers]
    bias = bias[:k_layers]
    w_sw = np.ascontiguousarray(
        W.transpose(2, 0, 1, 3).reshape(D, k_layers * R * D)).astype(BF16)
    root_sw = np.ascontiguousarray(
        root.transpose(1, 0, 2).reshape(D, k_layers * D)).astype(BF16)
    bias_in = bias.reshape(1, k_layers * D).astype(BF16)
    ident = np.eye(128, dtype=np.float32)
    iota = np.tile(np.arange(128, dtype=np.float32), (128, 1)).astype(BF16)

    # global bf16 table [trows, D]
    table = np.zeros((cfg.trows, D), dtype=BF16)
    for c in range(cfg.ncores):
        sl = x[c * ns:(c + 1) * ns].astype(BF16)
        rowpos = c * nsp + perms[c]
        table[rowpos] = sl

    in_maps = []
    for c in range(cfg.ncores):
        x_own = np.zeros((128, nsp), dtype=np.float32)
        inv_perm = np.full(nsp, -1, dtype=np.int64)
        for v in range(ns):
            inv_perm[perms[c][v]] = v
        for t in range(tpc):
            vv = inv_perm[t * 128:(t + 1) * 128]
            ok = vv >= 0
            x_own[ok, t * 128:(t + 1) * 128] = x[c * ns + vv[ok]]
        pc = per_core[c]
        im = dict(x_table=table, x_own=x_own, w_sw=w_sw, root_sw=root_sw,
                  bias_in=bias_in, ident_in=ident, iota_in=iota)
        for s in range(nstreams):
            if nseg[s]:
                im[f"idx{s}"] = pc[f"idx{s}"]
                im[f"cv{s}"] = pc[f"cv{s}"]
                im[f"ivc{s}"] = pc[f"ivc{s}"]
        in_maps.append(im)
    return in_maps


def _run(cfg, x, edge_index, edge_attr, basis, att, root, bias, prelu_a,
         k_layers=K, trace=False, n_iter=1):
    from concourse.bass_utils import run_bass_kernel_spmd

    sched, per_core, perms, inv_cnt = _preprocess(cfg, edge_index, edge_attr)
    nc = _build_program(cfg, sched, k_layers,
                        float(np.asarray(prelu_a).ravel()[0]), n_iter=n_iter)
    in_maps = _host_tensors(cfg, sched, per_core, perms, inv_cnt,
                            np.asarray(x, dtype=np.float32),
                            np.asarray(basis), np.asarray(att),
                            np.asarray(root), np.asarray(bias), k_layers)
    res = run_bass_kernel_spmd(nc, in_maps, core_ids=list(range(cfg.ncores)),
                               trace=trace)
    out = np.empty((cfg.n, D), dtype=np.float32)
    for c in range(cfg.ncores):
        rows = res.results[c]["out_own"]  # [nsp, D] permuted
        out[c * cfg.ns:(c + 1) * cfg.ns] = rows[perms[c]]
    return out, res


def kernel(x, edge_index, edge_attr, basis, att, root, bias, prelu_a):
    cfg = Cfg()
    out, _ = _run(cfg, x, edge_index, edge_attr, basis, att, root, bias,
                  prelu_a)
    return out
